# Initial kernel scaffold
#
"""Trainium2 Bass kernel for nn_Net_62801011802909.

Pipeline: embedder MLP (33->32->32->32) over [B,256,33], warm GRU (256
steps, hidden frozen past lengths_in), autoregressive decode (64 steps:
GRUCell on 2-d coords + MLP 256->64->64->2, outputs masked by
lengths_out).

Strategy: pure data parallel over batch (1024 -> 128 per core, 8 cores).
Everything on-device runs in "transposed" layout: hidden state h.T is
[hid-partitions, batch-cols] so the recurrent matmuls need no per-step
transposes. The lengths_in freeze is folded into the z-gate as an
additive +38 pre-activation (sigma(-pre-38) ~ 3e-15 -> h carries
through exactly to fp32 precision), delivered through an extra "mask"
channel appended to the embedded sequence. Biases ride the activation
instructions ([P,1] bias APs), so no ones-rows are needed.
"""

import numpy as np
from contextlib import ExitStack

N_CORES = 8
B_FULL, T_IN, T_OUT = 1024, 256, 64
IN_DIM, EMB, HID = 33, 32, 256
BC = B_FULL // N_CORES          # batch per core = 128
NTB = T_IN * BC                 # 32768 embedded columns (t-major: col = t*BC + b)
W = 2                           # warm steps per precomputed-xi window
NW = T_IN // W

_CACHE = {}


def _build_nc():
    import concourse.bass as bass  # noqa: F401
    import concourse.tile as tile
    from concourse import bacc, mybir
    from concourse.alu_op_type import AluOpType as ALU

    f32 = mybir.dt.float32
    AF = mybir.ActivationFunctionType

    nc = bacc.Bacc("TRN2", target_bir_lowering=False, debug=False,
                   num_devices=N_CORES)

    def din(name, shape):
        return nc.dram_tensor(name, shape, f32, kind="ExternalInput").ap()

    xT = din("xT", [IN_DIM, NTB])
    mrow = din("mrow", [1, NTB])
    dmask = din("dmask", [2, T_OUT * BC])
    lc0 = din("lc0", [2, BC])
    ew1 = din("ew1", [33, 32]); eb1 = din("eb1", [32, 1])
    ew2 = din("ew2", [32, 32]); eb2 = din("eb2", [32, 1])
    ew3 = din("ew3", [32, 32]); eb3 = din("eb3", [32, 1])
    gwi = din("gwi", [33, 768])
    gwh = din("gwh", [256, 768])
    gb = din("gb", [128, 6])
    gbhn = din("gbhn", [128, 2])
    cwi = din("cwi", [2, 768])
    cwh = din("cwh", [256, 768])
    cb = din("cb", [128, 6])
    cbhn = din("cbhn", [128, 2])
    pw1 = din("pw1", [256, 64]); pb1 = din("pb1", [64, 1])
    pw2 = din("pw2", [64, 64]); pb2 = din("pb2", [64, 1])
    pw3 = din("pw3", [64, 2]); pb3 = din("pb3", [2, 1])
    out = nc.dram_tensor("out", [2, T_OUT * BC], f32, kind="ExternalOutput").ap()
    xe = nc.dram_tensor("xe", [IN_DIM, NTB], f32, kind="Internal").ap()

    MM = nc.tensor.matmul
    ACT = nc.scalar.activation

    with tile.TileContext(nc) as tc, ExitStack() as ctx:
        wp = ctx.enter_context(tc.tile_pool(name="wp", bufs=1))
        hp = ctx.enter_context(tc.tile_pool(name="hp", bufs=3))
        lcp = ctx.enter_context(tc.tile_pool(name="lcp", bufs=3))

        def wtile(src_ap, shape, tag):
            t_ = wp.tile(shape, f32, tag=tag, name=tag)
            nc.sync.dma_start(t_[:], src_ap)
            return t_

        ew1_t = wtile(ew1[:], [33, 32], "ew1")
        eb1_t = wtile(eb1[:], [32, 1], "eb1")
        ew2_t = wtile(ew2[:], [32, 32], "ew2")
        eb2_t = wtile(eb2[:], [32, 1], "eb2")
        ew3_t = wtile(ew3[:], [32, 32], "ew3")
        eb3_t = wtile(eb3[:], [32, 1], "eb3")
        gwi_t = [wtile(gwi[:, c * 128:(c + 1) * 128], [33, 128], f"gwi{c}")
                 for c in range(6)]
        gwh_t = [[wtile(gwh[k * 128:(k + 1) * 128, m * 128:(m + 1) * 128],
                        [128, 128], f"gwh{k}_{m}") for m in range(6)]
                 for k in range(2)]
        gb_t = [wtile(gb[:, c:c + 1], [128, 1], f"gb{c}") for c in range(6)]
        gbhn_t = [wtile(gbhn[:, j:j + 1], [128, 1], f"gbhn{j}") for j in range(2)]
        cwi_t = [wtile(cwi[:, c * 128:(c + 1) * 128], [2, 128], f"cwi{c}")
                 for c in range(6)]
        cwh_t = [[wtile(cwh[k * 128:(k + 1) * 128, m * 128:(m + 1) * 128],
                        [128, 128], f"cwh{k}_{m}") for m in range(6)]
                 for k in range(2)]
        cb_t = [wtile(cb[:, c:c + 1], [128, 1], f"cb{c}") for c in range(6)]
        cbhn_t = [wtile(cbhn[:, j:j + 1], [128, 1], f"cbhn{j}") for j in range(2)]
        pw1_t = [wtile(pw1[k * 128:(k + 1) * 128, :], [128, 64], f"pw1{k}")
                 for k in range(2)]
        pb1_t = wtile(pb1[:], [64, 1], "pb1")
        pw2_t = wtile(pw2[:], [64, 64], "pw2")
        pb2_t = wtile(pb2[:], [64, 1], "pb2")
        pw3_t = wtile(pw3[:], [64, 2], "pw3")
        pb3_t = wtile(pb3[:], [2, 1], "pb3")

        # mask channel -> row 32 of the embedded-sequence DRAM buffer
        nc.sync.dma_start(xe[32:33, :], mrow[:])

        # ---- Phase A: embedder MLP, streamed through DRAM ----
        with ExitStack() as ec:
            eps = ec.enter_context(tc.tile_pool(name="eps", bufs=2, space="PSUM"))
            esb = ec.enter_context(tc.tile_pool(name="esb", bufs=3))
            CH = 512
            for i in range(NTB // CH):
                sl = slice(i * CH, (i + 1) * CH)
                xin = esb.tile([33, CH], f32, tag="xin", name="xin")
                nc.sync.dma_start(xin[:], xT[:, sl])
                p1 = eps.tile([32, CH], f32, tag="p1", name="p1")
                MM(p1[:], ew1_t[:], xin[:], start=True, stop=True)
                s1 = esb.tile([32, CH], f32, tag="s1", name="s1")
                ACT(s1[:], p1[:], AF.Relu, bias=eb1_t[:])
                p2 = eps.tile([32, CH], f32, tag="p2", name="p2")
                MM(p2[:], ew2_t[:], s1[:], start=True, stop=True)
                s2 = esb.tile([32, CH], f32, tag="s2", name="s2")
                ACT(s2[:], p2[:], AF.Relu, bias=eb2_t[:])
                p3 = eps.tile([32, CH], f32, tag="p3", name="p3")
                MM(p3[:], ew3_t[:], s2[:], start=True, stop=True)
                s3 = esb.tile([32, CH], f32, tag="s3", name="s3")
                ACT(s3[:], p3[:], AF.Identity, bias=eb3_t[:])
                nc.sync.dma_start(xe[0:32, sl], s3[:])

        # ---- Phase B: warm GRU, 256 steps ----
        h_cur = [None, None]
        with ExitStack() as wc:
            xps = wc.enter_context(tc.tile_pool(name="xps", bufs=2, space="PSUM"))
            gsb = wc.enter_context(tc.tile_pool(name="gsb", bufs=3))
            xesb = wc.enter_context(tc.tile_pool(name="xesb", bufs=3))
            for w in range(NW):
                wsl = slice(w * W * BC, (w + 1) * W * BC)
                xew = xesb.tile([33, W * BC], f32, tag="xew", name="xew")
                nc.sync.dma_start(xew[:], xe[:, wsl])
                ra = xps.tile([128, 512], f32, tag="ra", name="ra")
                za = xps.tile([128, 512], f32, tag="za", name="za")
                xn = xps.tile([128, 512], f32, tag="xn", name="xn")
                cc = xps.tile([128, 512], f32, tag="cc", name="cc")
                regions = [(ra, 0, 0), (ra, 1, 256), (za, 2, 0), (za, 3, 256),
                           (xn, 4, 0), (xn, 5, 256)]
                for tile_, c, off in regions:
                    is_n = c >= 4
                    if w == 0:
                        MM(tile_[:, off:off + 128], gwi_t[c][:], xew[:, 0:128],
                           start=True, stop=True)
                        MM(tile_[:, off + 128:off + 256], gwi_t[c][:],
                           xew[:, 128:256], start=True, stop=is_n)
                    else:
                        MM(tile_[:, off:off + 256], gwi_t[c][:], xew[:],
                           start=True, stop=is_n)
                for s in range(W):
                    t = w * W + s
                    r_ap = [ra[:, s * 128:(s + 1) * 128],
                            ra[:, 256 + s * 128:256 + (s + 1) * 128]]
                    z_ap = [za[:, s * 128:(s + 1) * 128],
                            za[:, 256 + s * 128:256 + (s + 1) * 128]]
                    x_ap = [xn[:, s * 128:(s + 1) * 128],
                            xn[:, 256 + s * 128:256 + (s + 1) * 128]]
                    c_ap = [cc[:, s * 128:(s + 1) * 128],
                            cc[:, 256 + s * 128:256 + (s + 1) * 128]]
                    h_new = []
                    if t == 0:
                        for j in range(2):
                            r_ = gsb.tile([128, 128], f32, tag=f"r{j}", name="r_")
                            ACT(r_[:], r_ap[j], AF.Sigmoid, bias=gb_t[j][:])
                            wz = gsb.tile([128, 128], f32, tag=f"wz{j}", name="wz")
                            ACT(wz[:], z_ap[j], AF.Sigmoid, bias=gb_t[2 + j][:],
                                scale=-1.0)
                            nn = gsb.tile([128, 128], f32, tag=f"nn{j}", name="nn")
                            nc.vector.scalar_tensor_tensor(
                                nn[:], r_[:], gbhn_t[j][:], x_ap[j],
                                ALU.mult, ALU.add)
                            n_ = gsb.tile([128, 128], f32, tag=f"n{j}", name="n_")
                            ACT(n_[:], nn[:], AF.Tanh, bias=gb_t[4 + j][:])
                            hn = hp.tile([128, 128], f32, tag=f"h{j}", name="hn")
                            nc.vector.tensor_mul(hn[:], wz[:], n_[:])
                            h_new.append(hn)
                    else:
                        for j in range(2):
                            MM(r_ap[j], gwh_t[0][j][:], h_cur[0][:],
                               start=False, stop=False)
                            MM(r_ap[j], gwh_t[1][j][:], h_cur[1][:],
                               start=False, stop=True)
                            MM(z_ap[j], gwh_t[0][2 + j][:], h_cur[0][:],
                               start=False, stop=False)
                            MM(z_ap[j], gwh_t[1][2 + j][:], h_cur[1][:],
                               start=False, stop=True)
                            MM(c_ap[j], gwh_t[0][4 + j][:], h_cur[0][:],
                               start=True, stop=False)
                            MM(c_ap[j], gwh_t[1][4 + j][:], h_cur[1][:],
                               start=False, stop=True)
                        for j in range(2):
                            r_ = gsb.tile([128, 128], f32, tag=f"r{j}", name="r_")
                            ACT(r_[:], r_ap[j], AF.Sigmoid, bias=gb_t[j][:])
                            wz = gsb.tile([128, 128], f32, tag=f"wz{j}", name="wz")
                            ACT(wz[:], z_ap[j], AF.Sigmoid, bias=gb_t[2 + j][:],
                                scale=-1.0)
                            tmp = gsb.tile([128, 128], f32, tag=f"tmp{j}", name="tmp")
                            nc.vector.scalar_tensor_tensor(
                                tmp[:], c_ap[j], gbhn_t[j][:], r_[:],
                                ALU.add, ALU.mult)
                            nn = gsb.tile([128, 128], f32, tag=f"nn{j}", name="nn")
                            nc.vector.tensor_add(nn[:], tmp[:], x_ap[j])
                            n_ = gsb.tile([128, 128], f32, tag=f"n{j}", name="n_")
                            ACT(n_[:], nn[:], AF.Tanh, bias=gb_t[4 + j][:])
                            d_ = gsb.tile([128, 128], f32, tag=f"d{j}", name="d_")
                            nc.vector.tensor_sub(d_[:], n_[:], h_cur[j][:])
                            e_ = gsb.tile([128, 128], f32, tag=f"e{j}", name="e_")
                            nc.vector.tensor_mul(e_[:], wz[:], d_[:])
                            hn = hp.tile([128, 128], f32, tag=f"h{j}", name="hn")
                            nc.vector.tensor_add(hn[:], h_cur[j][:], e_[:])
                            h_new.append(hn)
                    h_cur = h_new

        # ---- Phase C: autoregressive decode, 64 steps ----
        with ExitStack() as dc:
            dps = dc.enter_context(tc.tile_pool(name="dps", bufs=2, space="PSUM"))
            dsb = dc.enter_context(tc.tile_pool(name="dsb", bufs=3))
            dmp = dc.enter_context(tc.tile_pool(name="dmp", bufs=3))

            lc_prev = lcp.tile([2, BC], f32, tag="lc", name="lc_prev")
            nc.sync.dma_start(lc_prev[:], lc0[:])

            def emit_mlp(h_t, dm_t, xb_t, cb2_t, t):
                mp1 = xb_t[0:64, 256:384]
                MM(mp1, pw1_t[0][:], h_t[0][:], start=True, stop=False)
                MM(mp1, pw1_t[1][:], h_t[1][:], start=False, stop=True)
                y1 = dsb.tile([64, 128], f32, tag="y1", name="y1")
                ACT(y1[:], mp1, AF.Relu, bias=pb1_t[:])
                mp2 = xb_t[0:64, 384:512]
                MM(mp2, pw2_t[:], y1[:], start=True, stop=True)
                y2 = dsb.tile([64, 128], f32, tag="y2", name="y2")
                ACT(y2[:], mp2, AF.Relu, bias=pb2_t[:])
                mp3 = cb2_t[0:2, 256:384]
                MM(mp3, pw3_t[:], y2[:], start=True, stop=True)
                y3 = dsb.tile([2, 128], f32, tag="y3", name="y3")
                ACT(y3[:], mp3, AF.Identity, bias=pb3_t[:])
                lc_n = lcp.tile([2, 128], f32, tag="lc", name="lc_n")
                nc.vector.tensor_mul(lc_n[:], y3[:], dm_t[:])
                nc.sync.dma_start(out[:, t * 128:(t + 1) * 128], lc_n[:])
                return lc_n

            mlp_pend = None
            for t in range(T_OUT):
                dm = dmp.tile([2, 128], f32, tag="dm", name="dm")
                nc.sync.dma_start(dm[:], dmask[:, t * 128:(t + 1) * 128])
                ab = dps.tile([128, 512], f32, tag="ab", name="ab")
                xb = dps.tile([128, 512], f32, tag="xb", name="xb")
                cb2 = dps.tile([128, 512], f32, tag="cb", name="cb2")
                A = [ab[:, c * 128:(c + 1) * 128] for c in range(4)]
                XN = [xb[:, 0:128], xb[:, 128:256]]
                C = [cb2[:, 0:128], cb2[:, 128:256]]
                for j in range(2):
                    MM(A[j], cwh_t[0][j][:], h_cur[0][:], start=True, stop=False)
                    MM(A[j], cwh_t[1][j][:], h_cur[1][:], start=False, stop=False)
                    MM(A[2 + j], cwh_t[0][2 + j][:], h_cur[0][:],
                       start=True, stop=False)
                    MM(A[2 + j], cwh_t[1][2 + j][:], h_cur[1][:],
                       start=False, stop=False)
                    MM(C[j], cwh_t[0][4 + j][:], h_cur[0][:],
                       start=True, stop=False)
                    MM(C[j], cwh_t[1][4 + j][:], h_cur[1][:],
                       start=False, stop=True)
                if mlp_pend is not None:
                    lc_prev = emit_mlp(*mlp_pend)
                for c in range(4):
                    MM(A[c], cwi_t[c][:], lc_prev[:], start=False, stop=True)
                for c in (4, 5):
                    MM(XN[c - 4], cwi_t[c][:], lc_prev[:], start=True, stop=True)
                h_new = []
                for j in range(2):
                    r_ = dsb.tile([128, 128], f32, tag=f"dr{j}", name="r_")
                    ACT(r_[:], A[j], AF.Sigmoid, bias=cb_t[j][:])
                    wz = dsb.tile([128, 128], f32, tag=f"dwz{j}", name="wz")
                    ACT(wz[:], A[2 + j], AF.Sigmoid, bias=cb_t[2 + j][:],
                        scale=-1.0)
                    tmp = dsb.tile([128, 128], f32, tag=f"dtmp{j}", name="tmp")
                    nc.vector.scalar_tensor_tensor(
                        tmp[:], C[j], cbhn_t[j][:], r_[:], ALU.add, ALU.mult)
                    nn = dsb.tile([128, 128], f32, tag=f"dnn{j}", name="nn")
                    nc.vector.tensor_add(nn[:], tmp[:], XN[j])
                    n_ = dsb.tile([128, 128], f32, tag=f"dn{j}", name="n_")
                    ACT(n_[:], nn[:], AF.Tanh, bias=cb_t[4 + j][:])
                    d_ = dsb.tile([128, 128], f32, tag=f"dd{j}", name="d_")
                    nc.vector.tensor_sub(d_[:], n_[:], h_cur[j][:])
                    e_ = dsb.tile([128, 128], f32, tag=f"de{j}", name="e_")
                    nc.vector.tensor_mul(e_[:], wz[:], d_[:])
                    hn = hp.tile([128, 128], f32, tag=f"h{j}", name="hn")
                    nc.vector.tensor_add(hn[:], h_cur[j][:], e_[:])
                    h_new.append(hn)
                mlp_pend = (h_new, dm, xb, cb2, t)
                h_cur = h_new
            emit_mlp(*mlp_pend)

    nc.finalize()
    return nc


def _get_nc():
    if "nc" not in _CACHE:
        _CACHE["nc"] = _build_nc()
    return _CACHE["nc"]


def _prep_shared(inputs):
    f = np.float32

    def g(k):
        return np.asarray(inputs[k], f)

    gwiT = g("g_wi").T                                  # [32, 768]
    zrow = np.zeros((1, 768), f)
    zrow[0, 256:512] = 1.0
    gwi = np.ascontiguousarray(np.concatenate([gwiT, zrow], 0))

    def gate_bias(bi, bh):
        gb = np.zeros((128, 6), f)
        s = bi + bh
        gb[:, 0] = s[0:128]
        gb[:, 1] = s[128:256]
        gb[:, 2] = -s[256:384]
        gb[:, 3] = -s[384:512]
        gb[:, 4] = bi[512:640]
        gb[:, 5] = bi[640:768]
        bhn = np.ascontiguousarray(np.stack([bh[512:640], bh[640:768]], 1))
        return np.ascontiguousarray(gb), bhn

    gb, gbhn = gate_bias(g("g_bi"), g("g_bh"))
    cb, cbhn = gate_bias(g("c_bi"), g("c_bh"))

    c = np.ascontiguousarray
    return {
        "ew1": c(g("e_w1").T), "eb1": c(g("e_b1").reshape(32, 1)),
        "ew2": c(g("e_w2").T), "eb2": c(g("e_b2").reshape(32, 1)),
        "ew3": c(g("e_w3").T), "eb3": c(g("e_b3").reshape(32, 1)),
        "gwi": gwi, "gwh": c(g("g_wh").T), "gb": gb, "gbhn": gbhn,
        "cwi": c(g("c_wi").T), "cwh": c(g("c_wh").T), "cb": cb, "cbhn": cbhn,
        "pw1": c(g("p_w1").T), "pb1": c(g("p_b1").reshape(64, 1)),
        "pw2": c(g("p_w2").T), "pb2": c(g("p_b2").reshape(64, 1)),
        "pw3": c(g("p_w3").T), "pb3": c(g("p_b3").reshape(2, 1)),
    }


def _prep_core(inputs, ci):
    f = np.float32
    b0, b1 = ci * BC, (ci + 1) * BC
    in_seq = np.asarray(inputs["in_seq"], f)[b0:b1]
    li = np.asarray(inputs["lengths_in"]).astype(np.int64)[b0:b1]
    lo = np.asarray(inputs["lengths_out"]).astype(np.int64)[b0:b1]
    lc = np.asarray(inputs["last_cords"], f)[b0:b1]

    xT = np.ascontiguousarray(in_seq.transpose(2, 1, 0).reshape(IN_DIM, NTB))
    tt = np.arange(T_IN, dtype=np.int64)[:, None]
    mrow = np.ascontiguousarray(
        ((tt >= li[None, :]) * 38.0).astype(f).reshape(1, NTB))
    td = np.arange(T_OUT, dtype=np.int64)[:, None]
    dm1 = (td < lo[None, :]).astype(f).reshape(1, T_OUT * BC)
    dmask = np.ascontiguousarray(
        np.broadcast_to(dm1, (2, T_OUT * BC)).copy())
    lc0 = np.ascontiguousarray(lc.T)
    return {"xT": xT, "mrow": mrow, "dmask": dmask, "lc0": lc0}


def make_in_maps(inputs):
    shared = _prep_shared(inputs)
    in_maps = []
    for ci in range(N_CORES):
        m = dict(shared)
        m.update(_prep_core(inputs, ci))
        in_maps.append(m)
    return in_maps


def assemble(results):
    outs = []
    for ci in range(N_CORES):
        o = np.asarray(results[ci]["out"])
        outs.append(o.reshape(2, T_OUT, BC).transpose(2, 1, 0))
    return np.ascontiguousarray(np.concatenate(outs, 0)).astype(np.float32)


def kernel(**inputs):
    from concourse.bass_utils import run_bass_kernel_spmd
    nc = _get_nc()
    in_maps = make_in_maps(inputs)
    res = run_bass_kernel_spmd(nc, in_maps, list(range(N_CORES)))
    return assemble(res.results)


if __name__ == "__main__":
    nc = _get_nc()
    print("built ok")


# revision 14
# speedup vs baseline: 3360.6268x; 3360.6268x over previous
"""Trainium2 Bass kernel for nn_Net_62801011802909.

Pipeline: embedder MLP (33->32->32->32) over [B,256,33], warm GRU (256
steps, hidden frozen past lengths_in), autoregressive decode (64 steps:
GRUCell on 2-d coords + MLP 256->64->64->2, outputs masked by
lengths_out).

Strategy: pure data parallel over batch (1024 -> 128 per core, 8 cores).
Everything on-device runs in "transposed" layout: hidden state h.T is
[hid-partitions, batch-cols] so the recurrent matmuls need no per-step
transposes. The lengths_in freeze is folded into the z-gate as an
additive +38 pre-activation (sigma(-pre-38) ~ 3e-15 -> h carries
through exactly to fp32 precision), delivered through an extra "mask"
channel appended to the embedded sequence. Biases ride the activation
instructions ([P,1] bias APs), so no ones-rows are needed.
"""

import numpy as np
from contextlib import ExitStack

N_CORES = 8
B_FULL, T_IN, T_OUT = 1024, 256, 64
IN_DIM, EMB, HID = 33, 32, 256
BC = B_FULL // N_CORES          # batch per core = 128
NTB = T_IN * BC                 # 32768 embedded columns (t-major: col = t*BC + b)
W = 2                           # warm steps per precomputed-xi window
NW = T_IN // W

_CACHE = {}
DEBUG_DUMPS = False
DEBUG_TRUNC = None


def _build_nc():
    import concourse.bass as bass  # noqa: F401
    import concourse.tile as tile
    from concourse import bacc, mybir
    from concourse.alu_op_type import AluOpType as ALU

    f32 = mybir.dt.float32
    AF = mybir.ActivationFunctionType

    nc = bacc.Bacc("TRN2", target_bir_lowering=False, debug=False,
                   num_devices=N_CORES)

    def din(name, shape):
        return nc.dram_tensor(name, shape, f32, kind="ExternalInput").ap()

    xT = din("xT", [IN_DIM, NTB])
    mrow = din("mrow", [1, NTB])
    dmask = din("dmask", [2, T_OUT * BC])
    lc0 = din("lc0", [2, BC])
    ew1 = din("ew1", [33, 32]); eb1 = din("eb1", [32, 1])
    ew2 = din("ew2", [32, 32]); eb2 = din("eb2", [32, 1])
    ew3 = din("ew3", [32, 32]); eb3 = din("eb3", [32, 1])
    gwi = din("gwi", [33, 768])
    gwh = din("gwh", [256, 768])
    gb = din("gb", [128, 6])
    gbhn = din("gbhn", [128, 2])
    cwi = din("cwi", [2, 768])
    cwh = din("cwh", [256, 768])
    cb = din("cb", [128, 6])
    cbhn = din("cbhn", [128, 2])
    pw1 = din("pw1", [256, 64]); pb1 = din("pb1", [64, 1])
    pw2 = din("pw2", [64, 64]); pb2 = din("pb2", [64, 1])
    pw3 = din("pw3", [64, 2]); pb3 = din("pb3", [2, 1])
    out = nc.dram_tensor("out", [2, T_OUT * BC], f32, kind="ExternalOutput").ap()
    xe = nc.dram_tensor("xe", [IN_DIM, NTB], f32, kind="Internal").ap()
    if DEBUG_DUMPS:
        xdump = nc.dram_tensor("xdump", [IN_DIM, NTB], f32,
                               kind="ExternalOutput").ap()
        hdump = nc.dram_tensor("hdump", [256, BC], f32,
                               kind="ExternalOutput").ap()

    MM = nc.tensor.matmul
    ACT = nc.scalar.activation

    with tile.TileContext(nc) as tc, ExitStack() as ctx:
        wp = ctx.enter_context(tc.tile_pool(name="wp", bufs=1))
        hp = ctx.enter_context(tc.tile_pool(name="hp", bufs=3))
        lcp = ctx.enter_context(tc.tile_pool(name="lcp", bufs=3))

        def wtile(src_ap, shape, tag):
            t_ = wp.tile(shape, f32, tag=tag, name=tag)
            nc.sync.dma_start(t_[:], src_ap)
            return t_

        ew1_t = wtile(ew1[:], [33, 32], "ew1")
        eb1_t = wtile(eb1[:], [32, 1], "eb1")
        ew2_t = wtile(ew2[:], [32, 32], "ew2")
        eb2_t = wtile(eb2[:], [32, 1], "eb2")
        ew3_t = wtile(ew3[:], [32, 32], "ew3")
        eb3_t = wtile(eb3[:], [32, 1], "eb3")
        gwi_t = [wtile(gwi[:, c * 128:(c + 1) * 128], [33, 128], f"gwi{c}")
                 for c in range(6)]
        gwh_t = [[wtile(gwh[k * 128:(k + 1) * 128, m * 128:(m + 1) * 128],
                        [128, 128], f"gwh{k}_{m}") for m in range(6)]
                 for k in range(2)]
        gb_t = [wtile(gb[:, c:c + 1], [128, 1], f"gb{c}") for c in range(6)]
        gbhn_t = [wtile(gbhn[:, j:j + 1], [128, 1], f"gbhn{j}") for j in range(2)]
        cwi_t = [wtile(cwi[:, c * 128:(c + 1) * 128], [2, 128], f"cwi{c}")
                 for c in range(6)]
        cwh_t = [[wtile(cwh[k * 128:(k + 1) * 128, m * 128:(m + 1) * 128],
                        [128, 128], f"cwh{k}_{m}") for m in range(6)]
                 for k in range(2)]
        cb_t = [wtile(cb[:, c:c + 1], [128, 1], f"cb{c}") for c in range(6)]
        cbhn_t = [wtile(cbhn[:, j:j + 1], [128, 1], f"cbhn{j}") for j in range(2)]
        pw1_t = [wtile(pw1[k * 128:(k + 1) * 128, :], [128, 64], f"pw1{k}")
                 for k in range(2)]
        pb1_t = wtile(pb1[:], [64, 1], "pb1")
        pw2_t = wtile(pw2[:], [64, 64], "pw2")
        pb2_t = wtile(pb2[:], [64, 1], "pb2")
        pw3_t = wtile(pw3[:], [64, 2], "pw3")
        pb3_t = wtile(pb3[:], [2, 1], "pb3")

        # mask channel -> row 32 of the embedded-sequence DRAM buffer
        nc.sync.dma_start(xe[32:33, :], mrow[:])

        # ---- Phase A: embedder MLP, streamed through DRAM ----
        with ExitStack() as ec:
            eps = ec.enter_context(tc.tile_pool(name="eps", bufs=2, space="PSUM"))
            esb = ec.enter_context(tc.tile_pool(name="esb", bufs=3))
            CH = 512
            for i in range(NTB // CH):
                sl = slice(i * CH, (i + 1) * CH)
                xin = esb.tile([33, CH], f32, tag="xin", name="xin")
                nc.sync.dma_start(xin[:], xT[:, sl])
                p1 = eps.tile([32, CH], f32, tag="p1", name="p1")
                MM(p1[:], ew1_t[:], xin[:], start=True, stop=True)
                s1 = esb.tile([32, CH], f32, tag="s1", name="s1")
                ACT(s1[:], p1[:], AF.Relu, bias=eb1_t[:])
                p2 = eps.tile([32, CH], f32, tag="p2", name="p2")
                MM(p2[:], ew2_t[:], s1[:], start=True, stop=True)
                s2 = esb.tile([32, CH], f32, tag="s2", name="s2")
                ACT(s2[:], p2[:], AF.Relu, bias=eb2_t[:])
                p3 = eps.tile([32, CH], f32, tag="p3", name="p3")
                MM(p3[:], ew3_t[:], s2[:], start=True, stop=True)
                s3 = esb.tile([32, CH], f32, tag="s3", name="s3")
                ACT(s3[:], p3[:], AF.Identity, bias=eb3_t[:])
                nc.sync.dma_start(xe[0:32, sl], s3[:])

        # ---- Phase B: warm GRU, 256 steps ----
        # PSUM accumulation groups must stay contiguous per bank: a later
        # start=True on the same bank discards an open group's partials.
        # So each (gate, h-chunk) gets its own bank: rz0..rz3 and xn0/xn1
        # rotate window halves; cc double-buffers per step.
        h_cur = [None, None]
        with ExitStack() as wc:
            xps = wc.enter_context(tc.tile_pool(name="xps", bufs=1, space="PSUM"))
            ccp = wc.enter_context(tc.tile_pool(name="ccp", bufs=2, space="PSUM"))
            gsb = wc.enter_context(tc.tile_pool(name="gsb", bufs=3))
            xesb = wc.enter_context(tc.tile_pool(name="xesb", bufs=3))
            rz = [xps.tile([128, 512], f32, tag=f"rz{c}", name=f"rz{c}")
                  for c in range(4)]        # r j0, r j1, z j0, z j1
            xnt = [xps.tile([128, 512], f32, tag=f"xn{j}", name=f"xn{j}")
                   for j in range(2)]
            for w in range(NW):
                if DEBUG_TRUNC is not None and w * W >= DEBUG_TRUNC:
                    break
                half = (w % 2) * 256
                wsl = slice(w * W * BC, (w + 1) * W * BC)
                xew = xesb.tile([33, W * BC], f32, tag="xew", name="xew")
                nc.sync.dma_start(xew[:], xe[:, wsl])
                for j in range(2):
                    MM(xnt[j][:, half:half + 256], gwi_t[4 + j][:], xew[:],
                       start=True, stop=True)
                for c in range(4):
                    if w == 0:
                        MM(rz[c][:, 0:128], gwi_t[c][:], xew[:, 0:128],
                           start=True, stop=True)
                        MM(rz[c][:, 128:256], gwi_t[c][:], xew[:, 128:256],
                           start=True, stop=False)
                    else:
                        MM(rz[c][:, half:half + 256], gwi_t[c][:], xew[:],
                           start=True, stop=False)
                for s in range(W):
                    t = w * W + s
                    if DEBUG_TRUNC is not None and t >= DEBUG_TRUNC:
                        continue
                    col = half + s * 128
                    r_ap = [rz[0][:, col:col + 128], rz[1][:, col:col + 128]]
                    z_ap = [rz[2][:, col:col + 128], rz[3][:, col:col + 128]]
                    x_ap = [xnt[0][:, col:col + 128], xnt[1][:, col:col + 128]]
                    h_new = []
                    if t == 0:
                        for j in range(2):
                            r_ = gsb.tile([128, 128], f32, tag=f"r{j}", name="r_")
                            ACT(r_[:], r_ap[j], AF.Sigmoid, bias=gb_t[j][:])
                            wz = gsb.tile([128, 128], f32, tag=f"wz{j}", name="wz")
                            ACT(wz[:], z_ap[j], AF.Sigmoid, bias=gb_t[2 + j][:],
                                scale=-1.0)
                            nn = gsb.tile([128, 128], f32, tag=f"nn{j}", name="nn")
                            nc.vector.scalar_tensor_tensor(
                                nn[:], r_[:], gbhn_t[j][:], x_ap[j],
                                ALU.mult, ALU.add)
                            n_ = gsb.tile([128, 128], f32, tag=f"n{j}", name="n_")
                            ACT(n_[:], nn[:], AF.Tanh, bias=gb_t[4 + j][:])
                            hn = hp.tile([128, 128], f32, tag=f"h{j}", name="hn")
                            nc.vector.tensor_mul(hn[:], wz[:], n_[:])
                            h_new.append(hn)
                    else:
                        for c in range(4):
                            MM(rz[c][:, col:col + 128], gwh_t[0][c][:],
                               h_cur[0][:], start=False, stop=False)
                            MM(rz[c][:, col:col + 128], gwh_t[1][c][:],
                               h_cur[1][:], start=False, stop=True)
                        cc_s = ccp.tile([128, 256], f32, tag="cc", name="cc")
                        c_ap = [cc_s[:, 0:128], cc_s[:, 128:256]]
                        for j in range(2):
                            MM(c_ap[j], gwh_t[0][4 + j][:], h_cur[0][:],
                               start=True, stop=False)
                            MM(c_ap[j], gwh_t[1][4 + j][:], h_cur[1][:],
                               start=False, stop=True)
                        for j in range(2):
                            r_ = gsb.tile([128, 128], f32, tag=f"r{j}", name="r_")
                            ACT(r_[:], r_ap[j], AF.Sigmoid, bias=gb_t[j][:])
                            wz = gsb.tile([128, 128], f32, tag=f"wz{j}", name="wz")
                            ACT(wz[:], z_ap[j], AF.Sigmoid, bias=gb_t[2 + j][:],
                                scale=-1.0)
                            tmp = gsb.tile([128, 128], f32, tag=f"tmp{j}", name="tmp")
                            nc.vector.scalar_tensor_tensor(
                                tmp[:], c_ap[j], gbhn_t[j][:], r_[:],
                                ALU.add, ALU.mult)
                            nn = gsb.tile([128, 128], f32, tag=f"nn{j}", name="nn")
                            nc.vector.tensor_add(nn[:], tmp[:], x_ap[j])
                            n_ = gsb.tile([128, 128], f32, tag=f"n{j}", name="n_")
                            ACT(n_[:], nn[:], AF.Tanh, bias=gb_t[4 + j][:])
                            d_ = gsb.tile([128, 128], f32, tag=f"d{j}", name="d_")
                            nc.vector.tensor_sub(d_[:], n_[:], h_cur[j][:])
                            e_ = gsb.tile([128, 128], f32, tag=f"e{j}", name="e_")
                            nc.vector.tensor_mul(e_[:], wz[:], d_[:])
                            hn = hp.tile([128, 128], f32, tag=f"h{j}", name="hn")
                            nc.vector.tensor_add(hn[:], h_cur[j][:], e_[:])
                            h_new.append(hn)
                    h_cur = h_new

        if DEBUG_DUMPS:
            nc.sync.dma_start(xdump[:], xe[:])
            nc.sync.dma_start(hdump[0:128, :], h_cur[0][:])
            nc.sync.dma_start(hdump[128:256, :], h_cur[1][:])

        # ---- Phase C: autoregressive decode, 64 steps ----
        with ExitStack() as dc:
            dps = dc.enter_context(tc.tile_pool(name="dps", bufs=1, space="PSUM"))
            dbp = dc.enter_context(tc.tile_pool(name="dbp", bufs=2, space="PSUM"))
            dsb = dc.enter_context(tc.tile_pool(name="dsb", bufs=3))
            dmp = dc.enter_context(tc.tile_pool(name="dmp", bufs=3))
            at = [dps.tile([128, 512], f32, tag=f"a{c}", name=f"a{c}")
                  for c in range(4)]

            lc_prev = lcp.tile([2, BC], f32, tag="lc", name="lc_prev")
            nc.sync.dma_start(lc_prev[:], lc0[:])

            def emit_mlp(h_t, dm_t, xb_t, cb2_t, t):
                mp1 = xb_t[0:64, 256:384]
                MM(mp1, pw1_t[0][:], h_t[0][:], start=True, stop=False)
                MM(mp1, pw1_t[1][:], h_t[1][:], start=False, stop=True)
                y1 = dsb.tile([64, 128], f32, tag="y1", name="y1")
                ACT(y1[:], mp1, AF.Relu, bias=pb1_t[:])
                mp2 = xb_t[0:64, 384:512]
                MM(mp2, pw2_t[:], y1[:], start=True, stop=True)
                y2 = dsb.tile([64, 128], f32, tag="y2", name="y2")
                ACT(y2[:], mp2, AF.Relu, bias=pb2_t[:])
                mp3 = cb2_t[0:2, 256:384]
                MM(mp3, pw3_t[:], y2[:], start=True, stop=True)
                y3 = dsb.tile([2, 128], f32, tag="y3", name="y3")
                ACT(y3[:], mp3, AF.Identity, bias=pb3_t[:])
                lc_n = lcp.tile([2, 128], f32, tag="lc", name="lc_n")
                nc.vector.tensor_mul(lc_n[:], y3[:], dm_t[:])
                nc.sync.dma_start(out[:, t * 128:(t + 1) * 128], lc_n[:])
                return lc_n

            mlp_pend = None
            for t in range(T_OUT):
                rt = (t % 4) * 128
                dm = dmp.tile([2, 128], f32, tag="dm", name="dm")
                nc.sync.dma_start(dm[:], dmask[:, t * 128:(t + 1) * 128])
                A = [at[c][:, rt:rt + 128] for c in range(4)]
                for c in range(4):
                    MM(A[c], cwh_t[0][c][:], h_cur[0][:], start=True, stop=False)
                    MM(A[c], cwh_t[1][c][:], h_cur[1][:], start=False, stop=False)
                if mlp_pend is not None:
                    lc_prev = emit_mlp(*mlp_pend)
                for c in range(4):
                    MM(A[c], cwi_t[c][:], lc_prev[:], start=False, stop=True)
                xb = dbp.tile([128, 512], f32, tag="xb", name="xb")
                cb2 = dbp.tile([128, 512], f32, tag="cb", name="cb2")
                XN = [xb[:, 0:128], xb[:, 128:256]]
                C = [cb2[:, 0:128], cb2[:, 128:256]]
                for j in range(2):
                    MM(XN[j], cwi_t[4 + j][:], lc_prev[:], start=True, stop=True)
                    MM(C[j], cwh_t[0][4 + j][:], h_cur[0][:],
                       start=True, stop=False)
                    MM(C[j], cwh_t[1][4 + j][:], h_cur[1][:],
                       start=False, stop=True)
                h_new = []
                for j in range(2):
                    r_ = dsb.tile([128, 128], f32, tag=f"dr{j}", name="r_")
                    ACT(r_[:], A[j], AF.Sigmoid, bias=cb_t[j][:])
                    wz = dsb.tile([128, 128], f32, tag=f"dwz{j}", name="wz")
                    ACT(wz[:], A[2 + j], AF.Sigmoid, bias=cb_t[2 + j][:],
                        scale=-1.0)
                    tmp = dsb.tile([128, 128], f32, tag=f"dtmp{j}", name="tmp")
                    nc.vector.scalar_tensor_tensor(
                        tmp[:], C[j], cbhn_t[j][:], r_[:], ALU.add, ALU.mult)
                    nn = dsb.tile([128, 128], f32, tag=f"dnn{j}", name="nn")
                    nc.vector.tensor_add(nn[:], tmp[:], XN[j])
                    n_ = dsb.tile([128, 128], f32, tag=f"dn{j}", name="n_")
                    ACT(n_[:], nn[:], AF.Tanh, bias=cb_t[4 + j][:])
                    d_ = dsb.tile([128, 128], f32, tag=f"dd{j}", name="d_")
                    nc.vector.tensor_sub(d_[:], n_[:], h_cur[j][:])
                    e_ = dsb.tile([128, 128], f32, tag=f"de{j}", name="e_")
                    nc.vector.tensor_mul(e_[:], wz[:], d_[:])
                    hn = hp.tile([128, 128], f32, tag=f"h{j}", name="hn")
                    nc.vector.tensor_add(hn[:], h_cur[j][:], e_[:])
                    h_new.append(hn)
                mlp_pend = (h_new, dm, xb, cb2, t)
                h_cur = h_new
            emit_mlp(*mlp_pend)

    nc.finalize()
    return nc


def _get_nc():
    if "nc" not in _CACHE:
        _CACHE["nc"] = _build_nc()
    return _CACHE["nc"]


def _prep_shared(inputs):
    f = np.float32

    def g(k):
        return np.asarray(inputs[k], f)

    gwiT = g("g_wi").T                                  # [32, 768]
    zrow = np.zeros((1, 768), f)
    zrow[0, 256:512] = 1.0
    gwi = np.ascontiguousarray(np.concatenate([gwiT, zrow], 0))

    def gate_bias(bi, bh):
        gb = np.zeros((128, 6), f)
        s = bi + bh
        gb[:, 0] = s[0:128]
        gb[:, 1] = s[128:256]
        gb[:, 2] = -s[256:384]
        gb[:, 3] = -s[384:512]
        gb[:, 4] = bi[512:640]
        gb[:, 5] = bi[640:768]
        bhn = np.ascontiguousarray(np.stack([bh[512:640], bh[640:768]], 1))
        return np.ascontiguousarray(gb), bhn

    gb, gbhn = gate_bias(g("g_bi"), g("g_bh"))
    cb, cbhn = gate_bias(g("c_bi"), g("c_bh"))

    c = np.ascontiguousarray
    return {
        "ew1": c(g("e_w1").T), "eb1": c(g("e_b1").reshape(32, 1)),
        "ew2": c(g("e_w2").T), "eb2": c(g("e_b2").reshape(32, 1)),
        "ew3": c(g("e_w3").T), "eb3": c(g("e_b3").reshape(32, 1)),
        "gwi": gwi, "gwh": c(g("g_wh").T), "gb": gb, "gbhn": gbhn,
        "cwi": c(g("c_wi").T), "cwh": c(g("c_wh").T), "cb": cb, "cbhn": cbhn,
        "pw1": c(g("p_w1").T), "pb1": c(g("p_b1").reshape(64, 1)),
        "pw2": c(g("p_w2").T), "pb2": c(g("p_b2").reshape(64, 1)),
        "pw3": c(g("p_w3").T), "pb3": c(g("p_b3").reshape(2, 1)),
    }


def _prep_core(inputs, ci):
    f = np.float32
    b0, b1 = ci * BC, (ci + 1) * BC
    in_seq = np.asarray(inputs["in_seq"], f)[b0:b1]
    li = np.asarray(inputs["lengths_in"]).astype(np.int64)[b0:b1]
    lo = np.asarray(inputs["lengths_out"]).astype(np.int64)[b0:b1]
    lc = np.asarray(inputs["last_cords"], f)[b0:b1]

    xT = np.ascontiguousarray(in_seq.transpose(2, 1, 0).reshape(IN_DIM, NTB))
    tt = np.arange(T_IN, dtype=np.int64)[:, None]
    mrow = np.ascontiguousarray(
        ((tt >= li[None, :]) * 38.0).astype(f).reshape(1, NTB))
    td = np.arange(T_OUT, dtype=np.int64)[:, None]
    dm1 = (td < lo[None, :]).astype(f).reshape(1, T_OUT * BC)
    dmask = np.ascontiguousarray(
        np.broadcast_to(dm1, (2, T_OUT * BC)).copy())
    lc0 = np.ascontiguousarray(lc.T)
    return {"xT": xT, "mrow": mrow, "dmask": dmask, "lc0": lc0}


def make_in_maps(inputs):
    shared = _prep_shared(inputs)
    in_maps = []
    for ci in range(N_CORES):
        m = dict(shared)
        m.update(_prep_core(inputs, ci))
        in_maps.append(m)
    return in_maps


def assemble(results):
    outs = []
    for ci in range(N_CORES):
        o = np.asarray(results[ci]["out"])
        outs.append(o.reshape(2, T_OUT, BC).transpose(2, 1, 0))
    return np.ascontiguousarray(np.concatenate(outs, 0)).astype(np.float32)


def kernel(**inputs):
    from concourse.bass_utils import run_bass_kernel_spmd
    nc = _get_nc()
    in_maps = make_in_maps(inputs)
    res = run_bass_kernel_spmd(nc, in_maps, list(range(N_CORES)))
    return assemble(res.results)


if __name__ == "__main__":
    nc = _get_nc()
    print("built ok")
